# revision 7
# baseline (speedup 1.0000x reference)
"""Trainium2 Bass kernel for the NODE RK4 cell.

reference semantics:
    x_proj = x @ Wx.T + b                      # [B, U], constant
    f(s)   = tanh(x_proj + s @ Ws.T)
    6x RK4: k_i = 0.1 * f(...); s += (k1 + 2k2 + 2k3 + k4)/6

Strategy (pure data parallel, 8 cores, 8192 rows each):
  * Host transposes shards into [units, batch] layout so the contraction
    dim (units) lands on SBUF partitions; no on-device transposes at all.
  * Per core the batch is processed in 8 column-chunks of 1024. Each chunk
    keeps its pre-activation Z in a 2-bank PSUM tile for the entire
    6-unfold recurrence; 4 chunks are resident in PSUM at once so
    PE / ACT / DVE stay concurrently busy.
  * Per unfold: Z = Wxb@xa + Ws@s (fp32r matmuls, 1 cyc/row), then the RK
    stage inputs are built by accumulating small bf16 correction matmuls
    with host/device pre-scaled weights:
        z2 = z1 + 0.05*Ws@t1
        z3 = z2 + 0.05*Ws@t2 - 0.05*Ws@t1
        z4 = z3 + 0.10*Ws@t3 - 0.05*Ws@t2
    tanh runs on ScalarE straight out of PSUM, emitting bf16 t_i.
  * State update on VectorE in bf16 2x mode:
        u = t1+t4; v = t2+t3; u += v; u += v   ->  t1+t4+2(t2+t3)
        s = (u * 1/60) + s                     (fused scalar_tensor_tensor)
"""

import numpy as np
from contextlib import ExitStack

import concourse.bass as bass
import concourse.tile as tile
from concourse import bacc
from concourse import mybir
from concourse.bass_utils import run_bass_kernel_spmd

NCORES = 8
BATCH = 65536
BLOC = BATCH // NCORES  # 8192
U = 128                 # state units
D = 64                  # input dim
KA = D + 1              # augmented contraction (x rows + ones row for bias)
UNFOLDS = 6
DT = 0.1
C1 = DT / 6.0

CHUNK = 1024            # batch columns per PSUM-resident chunk
NMM = CHUNK // 512      # matmuls (512-wide) per chunk pass
NCHUNK = BLOC // CHUNK  # 8
PSUM_BUFS = 4           # chunks resident in PSUM simultaneously
F32 = mybir.dt.float32
F32R = mybir.dt.float32r
BF16 = mybir.dt.bfloat16
TANH = mybir.ActivationFunctionType.Tanh
ADD = mybir.AluOpType.add
MULT = mybir.AluOpType.mult


def build_module(bloc=BLOC, chunk=CHUNK):
    nmm = chunk // 512
    nchunk = bloc // chunk
    nc = bacc.Bacc("TRN2", target_bir_lowering=False)

    xa = nc.declare_dram_parameter("xa", [KA, bloc], F32R, isOutput=False)     # [x.T ; ones]
    st = nc.declare_dram_parameter("st", [U, bloc], F32R, isOutput=False)      # state.T
    wxb = nc.declare_dram_parameter("wxb", [KA, U], F32R, isOutput=False)      # [Wx.T ; b]
    wst = nc.declare_dram_parameter("wst", [U, U], F32R, isOutput=False)       # Ws.T
    out = nc.declare_dram_parameter("out", [U, bloc], F32R, isOutput=True)

    with ExitStack() as ctx:
        tc = ctx.enter_context(tile.TileContext(nc))
        const = ctx.enter_context(tc.tile_pool(name="const", bufs=1))
        spool = ctx.enter_context(tc.tile_pool(name="spool", bufs=PSUM_BUFS + 1))
        xpool = ctx.enter_context(tc.tile_pool(name="xpool", bufs=PSUM_BUFS + 1))
        tpool = ctx.enter_context(tc.tile_pool(name="tpool", bufs=PSUM_BUFS + 1))
        zpool = ctx.enter_context(tc.tile_pool(name="zpool", bufs=PSUM_BUFS, space="PSUM"))

        # constants: weights (fp32 masters + scaled bf16 copies)
        wxb_t = const.tile([KA, U], F32R)
        nc.sync.dma_start(out=wxb_t, in_=wxb[:, :])
        wst_t = const.tile([U, U], F32R)
        nc.sync.dma_start(out=wst_t, in_=wst[:, :])
        w05 = const.tile([U, U], BF16)
        nc.vector.tensor_scalar_mul(w05, wst_t.bitcast(F32), 0.05)
        w05n = const.tile([U, U], BF16)
        nc.vector.tensor_scalar_mul(w05n, wst_t.bitcast(F32), -0.05)
        w10 = const.tile([U, U], BF16)
        nc.vector.tensor_scalar_mul(w10, wst_t.bitcast(F32), 0.1)

        wxb_r = wxb_t
        wst_r = wst_t

        ngroup = (nchunk + PSUM_BUFS - 1) // PSUM_BUFS
        for g in range(ngroup):
            chunks = [c for c in range(g * PSUM_BUFS, min((g + 1) * PSUM_BUFS, nchunk))]
            s_t, xa_t, z = {}, {}, {}
            for c in chunks:
                s_t[c] = spool.tile([U, chunk], F32R, tag="s", name=f"s_{c}")
                nc.sync.dma_start(out=s_t[c], in_=st[:, c * chunk:(c + 1) * chunk])
                xa_t[c] = xpool.tile([KA, chunk], F32R, tag="xa", name=f"xa_{c}")
                nc.sync.dma_start(out=xa_t[c], in_=xa[:, c * chunk:(c + 1) * chunk])
                z[c] = zpool.tile([U, chunk], F32, tag="z", name=f"z_{c}")

            for n in range(UNFOLDS):
                last = n == UNFOLDS - 1
                for c in chunks:
                    zc, sc, xc = z[c], s_t[c], xa_t[c]
                    sc_r = sc
                    xc_r = xc
                    t = [tpool.tile([U, chunk], BF16, tag=f"t{i}", name=f"t{i}_{c}_{n}") for i in range(4)]

                    # Each RK stage closes its PSUM accumulation group
                    # (stop=True) before tanh reads it; later stages reopen
                    # with start=False + skip_group_check (stop is a sim-only
                    # flag; hardware accumulation is driven purely by start).
                    for j in range(nmm):
                        sl = slice(j * 512, (j + 1) * 512)
                        nc.tensor.matmul(zc[:, sl], wxb_r, xc_r[:, sl], start=True, stop=False)
                        nc.tensor.matmul(zc[:, sl], wst_r, sc_r[:, sl], start=False, stop=True)
                    nc.scalar.activation(out=t[0], in_=zc, func=TANH)

                    for j in range(nmm):
                        sl = slice(j * 512, (j + 1) * 512)
                        nc.tensor.matmul(zc[:, sl], w05, t[0][:, sl], start=False, stop=True,
                                         skip_group_check=True)
                    nc.scalar.activation(out=t[1], in_=zc, func=TANH)

                    for j in range(nmm):
                        sl = slice(j * 512, (j + 1) * 512)
                        nc.tensor.matmul(zc[:, sl], w05, t[1][:, sl], start=False, stop=False,
                                         skip_group_check=True)
                        nc.tensor.matmul(zc[:, sl], w05n, t[0][:, sl], start=False, stop=True,
                                         skip_group_check=True)
                    nc.scalar.activation(out=t[2], in_=zc, func=TANH)

                    for j in range(nmm):
                        sl = slice(j * 512, (j + 1) * 512)
                        nc.tensor.matmul(zc[:, sl], w10, t[2][:, sl], start=False, stop=False,
                                         skip_group_check=True)
                        nc.tensor.matmul(zc[:, sl], w05n, t[1][:, sl], start=False, stop=True,
                                         skip_group_check=True)
                    nc.scalar.activation(out=t[3], in_=zc, func=TANH)

                    # u = t1+t4; v = t2+t3; u += v; u += v  -> t1+t4+2(t2+t3)
                    u = tpool.tile([U, chunk], BF16, tag="u", name=f"u_{c}_{n}")
                    v = tpool.tile([U, chunk], BF16, tag="v", name=f"v_{c}_{n}")
                    nc.vector.tensor_tensor(out=u, in0=t[0], in1=t[3], op=ADD)
                    nc.vector.tensor_tensor(out=v, in0=t[1], in1=t[2], op=ADD)
                    nc.vector.tensor_tensor(out=u, in0=u, in1=v, op=ADD)
                    nc.vector.tensor_tensor(out=u, in0=u, in1=v, op=ADD)
                    # s = (u * 1/60) + s
                    nc.vector.scalar_tensor_tensor(
                        out=sc, in0=u, scalar=C1, in1=sc, op0=MULT, op1=ADD)
                    if last:
                        nc.sync.dma_start(out=out[:, c * chunk:(c + 1) * chunk], in_=sc)
    nc.compile()
    return nc


_NC_CACHE = {}


def _get_module():
    if "nc" not in _NC_CACHE:
        _NC_CACHE["nc"] = build_module()
    return _NC_CACHE["nc"]


def kernel(inputs, state, W, b):
    inputs = np.ascontiguousarray(np.asarray(inputs, dtype=np.float32))
    state = np.ascontiguousarray(np.asarray(state, dtype=np.float32))
    W = np.asarray(W, dtype=np.float32)
    b = np.asarray(b, dtype=np.float32)

    wxb = np.ascontiguousarray(np.vstack([W[:, :D].T, b[None, :]]))  # [65, 128]
    wst = np.ascontiguousarray(W[:, D:].T)                           # [128, 128]

    in_maps = []
    for c in range(NCORES):
        rows = slice(c * BLOC, (c + 1) * BLOC)
        xa_c = np.empty((KA, BLOC), dtype=np.float32)
        xa_c[:D] = inputs[rows].T
        xa_c[D] = 1.0
        st_c = np.ascontiguousarray(state[rows].T)
        in_maps.append({"xa": xa_c, "st": st_c, "wxb": wxb, "wst": wst})

    nc = _get_module()
    res = run_bass_kernel_spmd(nc, in_maps, core_ids=list(range(NCORES)))
    outs = [res.results[c]["out"] for c in range(NCORES)]
    full = np.concatenate(outs, axis=1).T  # [BATCH, U]
    full = np.ascontiguousarray(full, dtype=np.float32)
    return (full, full)


# revision 9
# speedup vs baseline: 14.6447x; 14.6447x over previous
"""Trainium2 Bass kernel for the NODE RK4 cell.

reference semantics:
    x_proj = x @ Wx.T + b                      # [B, U], constant
    f(s)   = tanh(x_proj + s @ Ws.T)
    6x RK4: k_i = 0.1 * f(...); s += (k1 + 2k2 + 2k3 + k4)/6

Strategy (pure data parallel, 8 cores, 8192 rows each):
  * Host transposes shards into [units, batch] layout so the contraction
    dim (units) lands on SBUF partitions; no on-device transposes at all.
  * Per core the batch is processed in 8 column-chunks of 1024. Each chunk
    keeps its pre-activation Z in a 2-bank PSUM tile for the entire
    6-unfold recurrence; 4 chunks are resident in PSUM at once so
    PE / ACT / DVE stay concurrently busy.
  * Per unfold: Z = Wxb@xa + Ws@s (fp32r matmuls, 1 cyc/row), then the RK
    stage inputs are built by accumulating small bf16 correction matmuls
    with host/device pre-scaled weights:
        z2 = z1 + 0.05*Ws@t1
        z3 = z2 + 0.05*Ws@t2 - 0.05*Ws@t1
        z4 = z3 + 0.10*Ws@t3 - 0.05*Ws@t2
    tanh runs on ScalarE straight out of PSUM, emitting bf16 t_i.
  * State update on VectorE in bf16 2x mode:
        u = t1+t4; v = t2+t3; u += v; u += v   ->  t1+t4+2(t2+t3)
        s = (u * 1/60) + s                     (fused scalar_tensor_tensor)
"""

import numpy as np
from contextlib import ExitStack

import concourse.bass as bass
import concourse.tile as tile
from concourse import bacc
from concourse import mybir
from concourse.bass_utils import run_bass_kernel_spmd

NCORES = 8
BATCH = 65536
BLOC = BATCH // NCORES  # 8192
U = 128                 # state units
D = 64                  # input dim
KA = D + 1              # augmented contraction (x rows + ones row for bias)
UNFOLDS = 6
DT = 0.1
C1 = DT / 6.0

CHUNK = 1024            # batch columns per PSUM-resident chunk
NMM = CHUNK // 512      # matmuls (512-wide) per chunk pass
NCHUNK = BLOC // CHUNK  # 8
PSUM_BUFS = 4           # chunks resident in PSUM simultaneously
F32 = mybir.dt.float32
F32R = mybir.dt.float32r
BF16 = mybir.dt.bfloat16
TANH = mybir.ActivationFunctionType.Tanh
ADD = mybir.AluOpType.add
MULT = mybir.AluOpType.mult


def build_module(bloc=BLOC, chunk=CHUNK, repeat=1):
    nmm = chunk // 512
    nchunk = bloc // chunk
    nc = bacc.Bacc("TRN2", target_bir_lowering=False)

    xa = nc.declare_dram_parameter("xa", [KA, bloc], F32R, isOutput=False)     # [x.T ; ones]
    st = nc.declare_dram_parameter("st", [U, bloc], F32R, isOutput=False)      # state.T
    wxb = nc.declare_dram_parameter("wxb", [KA, U], F32R, isOutput=False)      # [Wx.T ; b]
    wst = nc.declare_dram_parameter("wst", [U, U], F32R, isOutput=False)       # Ws.T
    out = nc.declare_dram_parameter("out", [U, bloc], F32R, isOutput=True)

    with ExitStack() as ctx:
        tc = ctx.enter_context(tile.TileContext(nc))
        const = ctx.enter_context(tc.tile_pool(name="const", bufs=1))
        spool = ctx.enter_context(tc.tile_pool(name="spool", bufs=6))
        xpool = ctx.enter_context(tc.tile_pool(name="xpool", bufs=6))
        tpool = ctx.enter_context(tc.tile_pool(name="tpool", bufs=6))
        zpool = ctx.enter_context(tc.tile_pool(name="zpool", bufs=PSUM_BUFS, space="PSUM"))

        # constants: weights (fp32 masters + scaled bf16 copies)
        wxb_t = const.tile([KA, U], F32R)
        nc.sync.dma_start(out=wxb_t, in_=wxb[:, :])
        wst_t = const.tile([U, U], F32R)
        nc.sync.dma_start(out=wst_t, in_=wst[:, :])
        w05 = const.tile([U, U], BF16)
        nc.vector.tensor_scalar_mul(w05, wst_t.bitcast(F32), 0.05)
        w05n = const.tile([U, U], BF16)
        nc.vector.tensor_scalar_mul(w05n, wst_t.bitcast(F32), -0.05)
        w10 = const.tile([U, U], BF16)
        nc.vector.tensor_scalar_mul(w10, wst_t.bitcast(F32), 0.1)

        # pre-load the tanh activation table while input DMAs run
        warm_t = const.tile([U, 2], BF16, name="warm_t")
        nc.scalar.activation(out=warm_t, in_=w05[:, 0:2], func=TANH)

        wxb_r = wxb_t
        wst_r = wst_t

        ngroup = (nchunk + PSUM_BUFS - 1) // PSUM_BUFS
        for r in range(repeat):
         for g in range(ngroup):
            chunks = [c for c in range(g * PSUM_BUFS, min((g + 1) * PSUM_BUFS, nchunk))]
            s_t, xa_t, z = {}, {}, {}
            for c in chunks:
                s_t[c] = spool.tile([U, chunk], F32R, tag="s", name=f"s_{r}_{c}")
                h = chunk // 2
                nc.sync.dma_start(out=s_t[c][:, :h], in_=st[:, c * chunk:c * chunk + h])
                nc.sync.dma_start(out=s_t[c][:, h:], in_=st[:, c * chunk + h:(c + 1) * chunk])
                xa_t[c] = xpool.tile([KA, chunk], F32R, tag="xa", name=f"xa_{r}_{c}")
                nc.sync.dma_start(out=xa_t[c][:, :h], in_=xa[:, c * chunk:c * chunk + h])
                nc.sync.dma_start(out=xa_t[c][:, h:], in_=xa[:, c * chunk + h:(c + 1) * chunk])
                z[c] = zpool.tile([U, chunk], F32, tag="z", name=f"z_{r}_{c}")

            for n in range(UNFOLDS):
                last = n == UNFOLDS - 1
                for c in chunks:
                    zc, sc, xc = z[c], s_t[c], xa_t[c]
                    sc_r = sc
                    xc_r = xc
                    t = [tpool.tile([U, chunk], BF16, tag=f"t{i}", name=f"t{i}_{r}_{c}_{n}") for i in range(4)]

                    # Each RK stage closes its PSUM accumulation group
                    # (stop=True) before tanh reads it; later stages reopen
                    # with start=False + skip_group_check (stop is a sim-only
                    # flag; hardware accumulation is driven purely by start).
                    for j in range(nmm):
                        sl = slice(j * 512, (j + 1) * 512)
                        nc.tensor.matmul(zc[:, sl], wxb_r, xc_r[:, sl], start=True, stop=False)
                        nc.tensor.matmul(zc[:, sl], wst_r, sc_r[:, sl], start=False, stop=True)
                    nc.scalar.activation(out=t[0], in_=zc, func=TANH)

                    for j in range(nmm):
                        sl = slice(j * 512, (j + 1) * 512)
                        nc.tensor.matmul(zc[:, sl], w05, t[0][:, sl], start=False, stop=True,
                                         skip_group_check=True)
                    nc.scalar.activation(out=t[1], in_=zc, func=TANH)

                    for j in range(nmm):
                        sl = slice(j * 512, (j + 1) * 512)
                        nc.tensor.matmul(zc[:, sl], w05, t[1][:, sl], start=False, stop=False,
                                         skip_group_check=True)
                        nc.tensor.matmul(zc[:, sl], w05n, t[0][:, sl], start=False, stop=True,
                                         skip_group_check=True)
                    nc.scalar.activation(out=t[2], in_=zc, func=TANH)

                    for j in range(nmm):
                        sl = slice(j * 512, (j + 1) * 512)
                        nc.tensor.matmul(zc[:, sl], w10, t[2][:, sl], start=False, stop=False,
                                         skip_group_check=True)
                        nc.tensor.matmul(zc[:, sl], w05n, t[1][:, sl], start=False, stop=True,
                                         skip_group_check=True)
                    nc.scalar.activation(out=t[3], in_=zc, func=TANH)

                    # u = t1+t4; v = t2+t3; u += v; u += v  -> t1+t4+2(t2+t3)
                    u = tpool.tile([U, chunk], BF16, tag="u", name=f"u_{r}_{c}_{n}")
                    v = tpool.tile([U, chunk], BF16, tag="v", name=f"v_{r}_{c}_{n}")
                    nc.vector.tensor_tensor(out=u, in0=t[0], in1=t[3], op=ADD)
                    nc.vector.tensor_tensor(out=v, in0=t[1], in1=t[2], op=ADD)
                    # u = u + 2v  ->  t1+t4+2(t2+t3)
                    nc.vector.scalar_tensor_tensor(
                        out=u, in0=v, scalar=2.0, in1=u, op0=MULT, op1=ADD)
                    # s = (u * 1/60) + s
                    nc.vector.scalar_tensor_tensor(
                        out=sc, in0=u, scalar=C1, in1=sc, op0=MULT, op1=ADD)
                    if last:
                        ho = chunk // 2
                        nc.sync.dma_start(out=out[:, c * chunk:c * chunk + ho], in_=sc[:, :ho])
                        nc.sync.dma_start(out=out[:, c * chunk + ho:(c + 1) * chunk], in_=sc[:, ho:])
    nc.compile()
    return nc


_NC_CACHE = {}


def _get_module():
    if "nc" not in _NC_CACHE:
        _NC_CACHE["nc"] = build_module()
    return _NC_CACHE["nc"]


def kernel(inputs, state, W, b):
    inputs = np.ascontiguousarray(np.asarray(inputs, dtype=np.float32))
    state = np.ascontiguousarray(np.asarray(state, dtype=np.float32))
    W = np.asarray(W, dtype=np.float32)
    b = np.asarray(b, dtype=np.float32)

    wxb = np.ascontiguousarray(np.vstack([W[:, :D].T, b[None, :]]))  # [65, 128]
    wst = np.ascontiguousarray(W[:, D:].T)                           # [128, 128]

    in_maps = []
    for c in range(NCORES):
        rows = slice(c * BLOC, (c + 1) * BLOC)
        xa_c = np.empty((KA, BLOC), dtype=np.float32)
        xa_c[:D] = inputs[rows].T
        xa_c[D] = 1.0
        st_c = np.ascontiguousarray(state[rows].T)
        in_maps.append({"xa": xa_c, "st": st_c, "wxb": wxb, "wst": wst})

    nc = _get_module()
    res = run_bass_kernel_spmd(nc, in_maps, core_ids=list(range(NCORES)))
    outs = [res.results[c]["out"] for c in range(NCORES)]
    full = np.concatenate(outs, axis=1).T  # [BATCH, U]
    full = np.ascontiguousarray(full, dtype=np.float32)
    return (full, full)
